# revision 1
# baseline (speedup 1.0000x reference)
"""Trainium2 Bass kernel for nn_NexusV2 (CentroidAddressableManifold.read).

Strategy: shard by *bucket* (not token). Tokens are routed host-side to the
core owning their bucket; each bucket's 32 slot rows are loaded exactly once
from HBM (vs. the reference's per-token gather => ~8x less memory traffic).

Device layout (per core, all shapes static at trace time):
  - tokens are packed into "instances" of <=16 tokens sharing one bucket
    (buckets with >16 tokens split into several instances)
  - groups of <=8 instances => <=128 token rows x <=256 slot columns
  - per group: PE computes scores = unified_query @ K^T (token-major,
    float32r), softmax + hard-match path on DVE/ACT, val = probs @ V on PE.

Host does only routing/permutation + transposed packing of the read-only
tables; all FLOPs of the reference (norms, dots, softmax, matches, matmuls)
run on device.
"""

import math
import sys
import types

import numpy as np

N_BUCKETS = 512
SPB = 32          # slots per bucket
TAU = 0.1
P_PAD = 16        # token rows per instance
IPG = 8           # instances per (full) group
N_CORES = 8
D = 1024
KCH = 8           # D / 128 contraction chunks
NEG = -30000.0    # additive mask value

_COMPILED = {}    # plan -> (nc, names)
_HOOK_DONE = False


# ----------------------------------------------------------------- utilities

def _install_ntff_hook():
    """Synthesize antenv.axon_hooks so trace=True can NTFF-profile (optional)."""
    global _HOOK_DONE
    if _HOOK_DONE or 'antenv.axon_hooks' in sys.modules:
        _HOOK_DONE = True
        return
    try:
        import antenv
        m = types.ModuleType('antenv.axon_hooks')
        _hook = [None]
        m.set_axon_ntff_profile_hook = lambda h: _hook.__setitem__(0, h)
        m.get_axon_ntff_profile_hook = lambda: _hook[0]
        sys.modules['antenv.axon_hooks'] = m
        antenv.axon_hooks = m
        if '/root/.axon_site' not in sys.path:
            sys.path.insert(0, '/root/.axon_site')
        from trn_agent_boot.trn_boot import _ntff_profile_via_ctypes
        m.set_axon_ntff_profile_hook(
            _ntff_profile_via_ctypes('/opt/axon/libaxon_pjrt.so'))
    except Exception:
        pass
    _HOOK_DONE = True


def _routing(tids_flat):
    """Return list of instances: (bucket_id, np.array of <=16 token indices)."""
    buckets = (tids_flat.astype(np.int64)) % N_BUCKETS
    order = np.argsort(buckets, kind='stable')
    counts = np.bincount(buckets, minlength=N_BUCKETS)
    cum = np.concatenate([[0], np.cumsum(counts)])
    instances = []
    for b in range(N_BUCKETS):
        c = int(counts[b])
        if c == 0:
            continue
        toks = order[cum[b]:cum[b] + c]
        for i in range(0, c, P_PAD):
            instances.append((b, toks[i:i + P_PAD]))
    return instances


def _plan(n_inst):
    i_core = (n_inst + N_CORES - 1) // N_CORES
    ngs, r = [], i_core
    while r > 0:
        ngs.append(min(IPG, r))
        r -= min(IPG, r)
    return i_core, tuple(ngs)


def _group_geom(ng):
    """Column geometry inside a group's kv block: KCH chunks of [K^T_k|a^T_k]
    (ns+ngp cols each), then nv V blocks of D cols. ngp = anchor dim padded
    even for fp32r matmul free-size restrictions."""
    ns = SPB * ng
    nv = 1 if ns <= 128 else 2
    ngp = ng + (ng % 2)
    return ns, nv, ngp, KCH * (ns + ngp) + nv * D


def _consts():
    r = np.arange(128)
    c256 = np.arange(256)
    valid = (c256[None, :] // SPB) == (r[:, None] // P_PAD)
    winadd = np.where(valid, 0.0, NEG).astype(np.float32)
    win01 = valid.astype(np.float32)
    oh8 = (np.arange(IPG)[None, :] == (r[:, None] // P_PAD)).astype(np.float32)
    oh8t_half = (0.5 * oh8.T).astype(np.float32)
    ident = np.eye(128, dtype=np.float32)
    return winadd, win01, oh8, oh8t_half, ident


def _pack_core(insts, ngs, q_flat, tids_flat, KT, V, slot_tids, CBT):
    """Build this core's input arrays. insts: list of (bucket, toks) or None."""
    i16 = sum(ngs) * P_PAD
    qr = np.zeros((i16, D), np.float32)
    trp = np.full((i16, 1), -1.0, np.float32)
    tidb = np.full((len(ngs), 2 * 128), -2.0, np.float32)
    tok_idx = np.full(i16, -1, np.int64)

    wtot = sum(_group_geom(ng)[3] for ng in ngs)
    kv = np.zeros((128, wtot), np.float32)

    col = 0
    row = 0
    ii = 0
    for g, ng in enumerate(ngs):
        ns, nv, ngp, wg = _group_geom(ng)
        group = insts[ii:ii + ng]
        ii += ng
        # slot ids (and bucket list) for this group
        slot_ids = np.zeros(ns, np.int64)
        real_slots = np.zeros(ns, bool)
        bucks = np.zeros(ng, np.int64)
        real_inst = np.zeros(ng, bool)
        for j, inst in enumerate(group):
            if inst is None:
                continue
            b, toks = inst
            bucks[j] = b
            real_inst[j] = True
            slot_ids[j * SPB:(j + 1) * SPB] = np.arange(b * SPB, (b + 1) * SPB)
            real_slots[j * SPB:(j + 1) * SPB] = True
            tidb[g, j * SPB:(j + 1) * SPB] = slot_tids[b * SPB:(b + 1) * SPB]
            r0 = row + j * P_PAD
            nt_real = len(toks)
            qr[r0:r0 + nt_real] = q_flat[toks]
            trp[r0:r0 + nt_real, 0] = tids_flat[toks]
            tok_idx[r0:r0 + nt_real] = toks
        # ka chunks [128, KCH, ns+ngp]: per chunk K^T slots then anchors
        ktg = KT[:, slot_ids].reshape(KCH, 128, ns) * real_slots[None, None, :]
        atp = np.zeros((KCH, 128, ngp), np.float32)
        atp[:, :, 0:ng] = CBT[:, bucks].reshape(KCH, 128, ng) \
            * real_inst[None, None, :]
        ka = np.concatenate([ktg, atp], axis=2)        # [KCH, 128, ns+ngp]
        kv[:, col:col + KCH * (ns + ngp)] = \
            ka.transpose(1, 0, 2).reshape(128, KCH * (ns + ngp))
        c = col + KCH * (ns + ngp)
        # V blocks
        vb = V[slot_ids] * real_slots[:, None]          # [ns, D]
        for h in range(nv):
            nsh = min(128, ns - h * 128)
            kv[0:nsh, c:c + D] = vb[h * 128:h * 128 + nsh]
            c += D
        col += wg
        row += ng * P_PAD
    return dict(qr=qr, kv=kv, tidb=tidb, trp=trp), tok_idx


# ------------------------------------------------------------- device kernel

def _build_nc(ngs, i16):
    from concourse import bacc, mybir, tile

    F32 = mybir.dt.float32
    F32R = mybir.dt.float32r
    AL = mybir.AluOpType
    AF = mybir.ActivationFunctionType
    X = mybir.AxisListType.X

    geoms = [_group_geom(ng) for ng in ngs]
    wtot = sum(g[3] for g in geoms)
    n_groups = len(ngs)
    kcols = np.concatenate([[0], np.cumsum([g[3] for g in geoms])])
    rows = np.concatenate([[0], np.cumsum([ng * P_PAD for ng in ngs])])

    nc = bacc.Bacc(trn_type="TRN2", target_bir_lowering=False, debug=False)
    d_qr = nc.dram_tensor("qr", [i16, D], F32, kind="ExternalInput").ap()
    d_kv = nc.dram_tensor("kv", [128, wtot], F32R, kind="ExternalInput").ap()
    d_tidb = nc.dram_tensor("tidb", [n_groups, 256], F32, kind="ExternalInput").ap()
    d_trp = nc.dram_tensor("trp", [i16, 1], F32, kind="ExternalInput").ap()
    d_winadd = nc.dram_tensor("winadd", [128, 256], F32, kind="ExternalInput").ap()
    d_win01 = nc.dram_tensor("win01", [128, 256], F32, kind="ExternalInput").ap()
    d_oh8 = nc.dram_tensor("oh8", [128, IPG], F32, kind="ExternalInput").ap()
    d_oh8t = nc.dram_tensor("oh8t", [IPG, 128], F32R, kind="ExternalInput").ap()
    d_ident = nc.dram_tensor("identw", [128, 128], F32, kind="ExternalInput").ap()
    d_out = nc.dram_tensor("outp", [i16, D], F32, kind="ExternalOutput").ap()

    with tile.TileContext(nc) as tc:
        with tc.tile_pool(name="const", bufs=1) as pc, \
             tc.tile_pool(name="kvp", bufs=4) as pkv, \
             tc.tile_pool(name="io", bufs=3) as pio, \
             tc.tile_pool(name="wk", bufs=2) as pw, \
             tc.tile_pool(name="ps", bufs=1, space="PSUM") as pp, \
             tc.tile_pool(name="ps2", bufs=2, space="PSUM") as pp2:

            winadd = pc.tile([128, 256], F32)
            win01 = pc.tile([128, 256], F32)
            oh8 = pc.tile([128, IPG], F32)
            oh8t = pc.tile([IPG, 128], F32R)
            ident = pc.tile([128, 128], F32)
            nc.sync.dma_start(winadd[:], d_winadd)
            nc.sync.dma_start(win01[:], d_win01)
            nc.sync.dma_start(oh8[:], d_oh8)
            nc.sync.dma_start(oh8t[:], d_oh8t)
            nc.sync.dma_start(ident[:], d_ident)
            eps24 = pc.tile([128, 1], F32)
            nc.gpsimd.memset(eps24[:], 1e-24)

            for g, ng in enumerate(ngs):
                ns, nv, ngp, wg = geoms[g]
                nt = ng * P_PAD
                nsp = ns + ngp
                col = kcols[g]

                kv_t = pkv.tile([128, 4160], F32R, tag="kv")
                nc.sync.dma_start(kv_t[:, 0:wg], d_kv[:, col:col + wg])
                ka = kv_t[:, 0:KCH * nsp].rearrange("p (k s) -> p k s", k=KCH)
                q_t = pio.tile([128, D], F32, tag="q")
                nc.scalar.dma_start(q_t[0:nt, :], d_qr[rows[g]:rows[g] + nt, :])
                tidb_t = pio.tile([128, 256], F32, tag="tidb")
                nc.sync.dma_start(tidb_t[0:nt, 0:ns],
                                  d_tidb[g:g + 1, 0:ns].to_broadcast((nt, ns)))
                tr_t = pio.tile([128, 1], F32, tag="tr")
                nc.scalar.dma_start(tr_t[0:nt, :], d_trp[rows[g]:rows[g] + nt, :])

                # --- normalize queries: qs = 0.5 * q / ||q||
                sq = pw.tile([128, D], F32, tag="sq")
                ssq = pw.tile([128, 1], F32, tag="ssq")
                nc.scalar.activation(sq[0:nt, :], q_t[0:nt, :], AF.Square,
                                     accum_out=ssq[0:nt, :])
                n2 = pw.tile([128, 1], F32, tag="n2")
                nc.scalar.activation(n2[0:nt, :], ssq[0:nt, :], AF.Sqrt,
                                     scale=4.0, bias=eps24[0:nt, :])
                rq2 = pw.tile([128, 1], F32, tag="rq2")
                nc.vector.reciprocal(rq2[0:nt, :], n2[0:nt, :])
                qs = pw.tile([128, D], F32, tag="qs")
                nc.vector.tensor_scalar(out=qs[0:nt, :], in0=q_t[0:nt, :],
                                        scalar1=rq2[0:nt, :], scalar2=None,
                                        op0=AL.mult)

                # --- transpose scaled queries -> qt [128d, KCH, nt] (f32r)
                qt = pw.tile([128, KCH, 128], F32R, tag="qt")
                for hb in range(2):
                    qth = pp2.tile([128, 512], F32, tag="qth")
                    for k in range(4):
                        kk = hb * 4 + k
                        nc.tensor.transpose(
                            qth[:, k * 128:k * 128 + nt],
                            qs[0:nt, kk * 128:(kk + 1) * 128],
                            ident[0:nt, 0:nt])
                    nc.vector.tensor_copy(
                        qt[:, hb * 4:(hb + 1) * 4, 0:nt],
                        qth[:].rearrange("p (k t) -> p k t", k=4)[:, :, 0:nt])

                # --- anchor-dot table a0t = a.K  [ngp, ns]
                a0t_ps = pp.tile([IPG, 256], F32, tag="a0t")
                for k in range(KCH):
                    nc.tensor.matmul(a0t_ps[0:ngp, 0:ns], ka[:, k, ns:nsp],
                                     ka[:, k, 0:ns], start=(k == 0),
                                     stop=(k == KCH - 1))
                a0t = pw.tile([IPG, 256], F32R, tag="a0tsb")
                nc.vector.tensor_copy(a0t[0:ngp, 0:ns], a0t_ps[0:ngp, 0:ns])

                # --- scores(+qa cols)+blend: [qn'.K | qn'.a] + 0.5*a.K
                sc_ps = pp.tile([128, 264], F32, tag="sc")
                for k in range(KCH):
                    nc.tensor.matmul(sc_ps[0:nt, 0:nsp], qt[:, k, 0:nt],
                                     ka[:, k, 0:nsp], start=(k == 0),
                                     stop=False)
                nc.tensor.matmul(sc_ps[0:nt, 0:ns], oh8t[0:ngp, 0:nt],
                                 a0t[0:ngp, 0:ns], start=False, stop=True)

                # --- rw = 1/|W| from qa cols; rw10 = rw/tau
                qasc = pw.tile([128, IPG], F32, tag="qasc")
                qa1 = pw.tile([128, 1], F32, tag="qa1")
                nc.vector.tensor_tensor(out=qasc[0:nt, 0:ngp],
                                        in0=sc_ps[0:nt, ns:nsp],
                                        in1=oh8[0:nt, 0:ngp], op=AL.mult)
                nc.vector.reduce_sum(qa1[0:nt, :], qasc[0:nt, 0:ngp], axis=X)
                w2 = pw.tile([128, 1], F32, tag="w2")
                nc.vector.tensor_scalar(out=w2[0:nt, :], in0=qa1[0:nt, :],
                                        scalar1=0.5, scalar2=None, op0=AL.add)
                wn = pw.tile([128, 1], F32, tag="wn")
                nc.scalar.activation(wn[0:nt, :], w2[0:nt, :], AF.Sqrt)
                rw = pw.tile([128, 1], F32, tag="rw")
                nc.vector.reciprocal(rw[0:nt, :], wn[0:nt, :])
                rw10 = pw.tile([128, 1], F32, tag="rw10")
                nc.vector.tensor_scalar(out=rw10[0:nt, :], in0=rw[0:nt, :],
                                        scalar1=1.0 / TAU, scalar2=None,
                                        op0=AL.mult)

                # --- masked scores, softmax with rw/tau in Exp scale
                sc = pw.tile([128, 256], F32, tag="scsb")
                nc.vector.tensor_tensor(out=sc[0:nt, 0:ns],
                                        in0=sc_ps[0:nt, 0:ns],
                                        in1=winadd[0:nt, 0:ns], op=AL.add)
                negmax = pw.tile([128, 1], F32, tag="negmax")
                nc.vector.reduce_max(negmax[0:nt, :], sc[0:nt, 0:ns], axis=X,
                                     negate=True)
                ebias = pw.tile([128, 1], F32, tag="ebias")
                nc.vector.tensor_tensor(out=ebias[0:nt, :], in0=negmax[0:nt, :],
                                        in1=rw10[0:nt, :], op=AL.mult)
                ex = pw.tile([128, 256], F32, tag="ex")
                esum = pw.tile([128, 1], F32, tag="esum")
                nc.scalar.activation(ex[0:nt, 0:ns], sc[0:nt, 0:ns], AF.Exp,
                                     bias=ebias[0:nt, :], scale=rw10[0:nt, :],
                                     accum_out=esum[0:nt, :])
                rsum = pw.tile([128, 1], F32, tag="rsum")
                nc.vector.reciprocal(rsum[0:nt, :], esum[0:nt, :])

                # --- hard match path
                match = pw.tile([128, 256], F32, tag="match")
                msum = pw.tile([128, 1], F32, tag="msum")
                nc.vector.scalar_tensor_tensor(
                    out=match[0:nt, 0:ns], in0=tidb_t[0:nt, 0:ns],
                    scalar=tr_t[0:nt, :], in1=win01[0:nt, 0:ns],
                    op0=AL.is_equal, op1=AL.mult, accum_out=msum[0:nt, :])
                mden = pw.tile([128, 1], F32, tag="mden")
                nc.vector.tensor_scalar(out=mden[0:nt, :], in0=msum[0:nt, :],
                                        scalar1=1e-9, scalar2=None, op0=AL.add)
                mrec = pw.tile([128, 1], F32, tag="mrec")
                nc.vector.reciprocal(mrec[0:nt, :], mden[0:nt, :])
                nohas = pw.tile([128, 1], F32, tag="nohas")
                nc.vector.tensor_scalar(out=nohas[0:nt, :], in0=msum[0:nt, :],
                                        scalar1=0.0, scalar2=None, op0=AL.is_le)
                hard = pw.tile([128, 256], F32, tag="hard")
                nc.vector.tensor_scalar(out=hard[0:nt, 0:ns],
                                        in0=match[0:nt, 0:ns],
                                        scalar1=mrec[0:nt, :], scalar2=None,
                                        op0=AL.mult)
                rs_nh = pw.tile([128, 1], F32, tag="rs_nh")
                nc.vector.tensor_tensor(out=rs_nh[0:nt, :], in0=rsum[0:nt, :],
                                        in1=nohas[0:nt, :], op=AL.mult)
                probs = pw.tile([128, 256], F32, tag="probs")
                nc.vector.scalar_tensor_tensor(
                    out=probs[0:nt, 0:ns], in0=ex[0:nt, 0:ns],
                    scalar=rs_nh[0:nt, :], in1=hard[0:nt, 0:ns],
                    op0=AL.mult, op1=AL.add)

                # --- probs^T, then val = probs @ V
                pt_ps = pp.tile([128, 264], F32, tag="pt")
                for h in range(nv):
                    nsh = min(128, ns - h * 128)
                    nc.tensor.transpose(pt_ps[0:nsh, h * 128:h * 128 + nt],
                                        probs[0:nt, h * 128:h * 128 + nsh],
                                        ident[0:nt, 0:nt])
                pt = pw.tile([128, 2, 128], F32R, tag="ptsb")
                for h in range(nv):
                    nsh = min(128, ns - h * 128)
                    nc.vector.tensor_copy(pt[0:nsh, h, 0:nt],
                                          pt_ps[0:nsh, h * 128:h * 128 + nt])
                pv = pp.tile([128, D], F32, tag="pv")
                for j in range(2):
                    for h in range(nv):
                        nsh = min(128, ns - h * 128)
                        nc.tensor.matmul(
                            pv[0:nt, j * 512:(j + 1) * 512],
                            pt[0:nsh, h, 0:nt],
                            kv_t[0:nsh, KCH * nsp + h * D + j * 512:
                                 KCH * nsp + h * D + (j + 1) * 512],
                            start=(h == 0), stop=(h == nv - 1))
                out_sb = pw.tile([128, D], F32, tag="out_sb")
                nc.vector.tensor_copy(out_sb[0:nt, :], pv[0:nt, :])
                nc.scalar.dma_start(d_out[rows[g]:rows[g] + nt, :],
                                    out_sb[0:nt, :])
    nc.compile()
    return nc


# ------------------------------------------------------------------ emulator

def _emulate_core(ins, ngs):
    """Numpy emulation of the device kernel (fp32), for validation."""
    qr, kv, tidb, trp = ins["qr"], ins["kv"], ins["tidb"], ins["trp"]
    i16 = qr.shape[0]
    out = np.zeros((i16, D), np.float32)
    winadd, win01, oh8, oh8t, _ = _consts()
    col = row = 0
    for g, ng in enumerate(ngs):
        ns, nv, ngp, wg = _group_geom(ng)
        nt = ng * P_PAD
        ka = kv[:, col:col + KCH * (ns + ngp)].reshape(128, KCH, ns + ngp)
        ktg = ka[:, :, 0:ns]
        atp = ka[:, :, ns:ns + ng]
        voff = col + KCH * (ns + ngp)
        vb = np.zeros((ns, D), np.float32)
        for h in range(nv):
            nsh = min(128, ns - h * 128)
            vb[h * 128:h * 128 + nsh] = kv[0:nsh, voff + h * D:voff + (h + 1) * D]

        q = qr[row:row + nt]
        ssq = (q * q).sum(-1, keepdims=True)
        rq2 = 1.0 / np.sqrt(4 * ssq + 1e-24)
        qn = q * rq2                                   # 0.5 * normalized
        KT = ktg.transpose(1, 0, 2).reshape(D, ns)     # [D, ns]
        AT = atp.transpose(1, 0, 2).reshape(D, ng)     # [D, ng]
        a0t = AT.T @ KT                                # [ng, ns]
        sc_ps = qn @ KT + (0.5 * oh8[0:nt, 0:ng]) @ a0t
        qa1 = ((qn @ AT) * oh8[0:nt, 0:ng]).sum(-1, keepdims=True)
        rw = 1.0 / np.sqrt(qa1 + 0.5)
        sc = sc_ps * rw + winadd[0:nt, 0:ns]
        m = sc.max(-1, keepdims=True)
        ex = np.exp((sc - m) / TAU)
        esum = ex.sum(-1, keepdims=True)
        match = (tidb[g, 0:ns][None, :] == trp[row:row + nt]) * win01[0:nt, 0:ns]
        msum = match.sum(-1, keepdims=True)
        nohas = (msum <= 0).astype(np.float32)
        hard = match / (msum + 1e-9)
        probs = ex * (nohas / esum) + hard
        out[row:row + nt] = probs.astype(np.float32) @ vb
        col += wg
        row += nt
    return out


# -------------------------------------------------------------------- kernel

def kernel(query_emb, tids, slot_keys, slot_values, slot_tids,
           centroid_codebook, _emulate=False, _trace=False):
    B, T, _ = query_emb.shape
    BT = B * T
    q_flat = np.ascontiguousarray(query_emb.reshape(BT, D), np.float32)
    tids_flat = np.asarray(tids).reshape(BT)
    st = np.asarray(slot_tids).astype(np.float32)
    KT = np.ascontiguousarray(np.asarray(slot_keys, np.float32).T)     # [D, S]
    V = np.asarray(slot_values, np.float32)
    CBT = np.ascontiguousarray(np.asarray(centroid_codebook, np.float32).T)

    instances = _routing(tids_flat)
    i_core, ngs = _plan(len(instances))
    padded = instances + [None] * (i_core * N_CORES - len(instances))
    i16 = i_core * P_PAD

    winadd, win01, oh8, oh8t, ident = _consts()
    in_maps, tok_idxs = [], []
    for c in range(N_CORES):
        ins, tok_idx = _pack_core(padded[c * i_core:(c + 1) * i_core], ngs,
                                  q_flat, tids_flat, KT, V, st, CBT)
        ins.update(winadd=winadd, win01=win01, oh8=oh8, oh8t=oh8t,
                   identw=ident)
        in_maps.append(ins)
        tok_idxs.append(tok_idx)

    out_flat = np.zeros((BT, D), np.float32)
    if _emulate:
        for c in range(N_CORES):
            o = _emulate_core(in_maps[c], ngs)
            valid = tok_idxs[c] >= 0
            out_flat[tok_idxs[c][valid]] = o[valid]
        return out_flat.reshape(B, T, D).astype(np.float32)

    _install_ntff_hook()
    from concourse import bass_utils
    key = (ngs, i16)
    if key not in _COMPILED:
        _COMPILED[key] = _build_nc(ngs, i16)
    nc = _COMPILED[key]
    res = bass_utils.run_bass_kernel_spmd(
        nc, in_maps, core_ids=list(range(N_CORES)), trace=_trace)
    for c in range(N_CORES):
        o = res.results[c]["outp"]
        valid = tok_idxs[c] >= 0
        out_flat[tok_idxs[c][valid]] = o[valid]
    out = out_flat.reshape(B, T, D).astype(np.float32)
    if _trace:
        kernel._last_exec_time_ns = res.exec_time_ns
        kernel._last_results = res
    return out



# revision 17
# speedup vs baseline: 1.4581x; 1.4581x over previous
"""Trainium2 Bass kernel for nn_NexusV2 (CentroidAddressableManifold.read).

Strategy: shard by *bucket*. Tokens are routed host-side to the core owning
their bucket; each occupied bucket's 32 slot rows stream from HBM exactly
once, in fp16 (vs. the reference's per-token f32 gather).

Device layout (per core, all shapes static at trace time):
  - 8 groups, each holding <=8 buckets; tokens of a bucket occupy a
    contiguous run of rows (no fixed padding); rows per group NT[g] is the
    max over cores for that group slot (SPMD shares one NEFF).
  - per group: PE computes u = q@[K|a] (fp16, token-major), one blend
    matmul adds the (window-masked) anchor-dot table, softmax + hard-match
    on DVE/ACT (Scalar runs only Rsqrt/Exp tables), val = probs @ V on PE.
  - |q| is folded into the softmax scale: no on-device q normalization or
    transposes (host supplies raw q both row-major and D-major).
  - hard-match windowing is folded into the tid encoding host-side
    (tid + window*2^17, exact in f32), so no mask tiles are built.

Host does only routing/permutation + packing of the read-only tables; all
FLOPs of the reference (norms, dots, softmax, matches, matmuls) run on
device.
"""

import sys
import types

import numpy as np

N_BUCKETS = 512
SPB = 32          # slots per bucket
TAU = 0.1
BPG = 8           # buckets per group
NGRP = 8          # groups per core
N_CORES = 8
D = 1024
KCH = 8           # D / 128 contraction chunks
NS = SPB * BPG    # 256 slot columns per group
NSP = NS + BPG
NEG = -30000.0    # additive mask value
WENC = 131072.0   # 2^17 window encoding for tid match

_COMPILED = {}    # plan -> nc
_HOOK_DONE = False


# ----------------------------------------------------------------- utilities

def _install_ntff_hook():
    """Synthesize antenv.axon_hooks so trace=True can NTFF-profile (optional)."""
    global _HOOK_DONE
    if _HOOK_DONE or 'antenv.axon_hooks' in sys.modules:
        _HOOK_DONE = True
        return
    try:
        import antenv
        m = types.ModuleType('antenv.axon_hooks')
        _hook = [None]
        m.set_axon_ntff_profile_hook = lambda h: _hook.__setitem__(0, h)
        m.get_axon_ntff_profile_hook = lambda: _hook[0]
        sys.modules['antenv.axon_hooks'] = m
        antenv.axon_hooks = m
        if '/root/.axon_site' not in sys.path:
            sys.path.insert(0, '/root/.axon_site')
        from trn_agent_boot.trn_boot import _ntff_profile_via_ctypes
        m.set_axon_ntff_profile_hook(
            _ntff_profile_via_ctypes('/opt/axon/libaxon_pjrt.so'))
    except Exception:
        pass
    _HOOK_DONE = True


def _routing(tids_flat):
    """Pack occupied buckets into NGRP*N_CORES bins (<=BPG buckets each),
    LPT-balanced by token count. Returns (bins, NT, tok_of_bucket) where
    bins[g][c] = bucket ids for core c / group-slot g, NT[g] = padded row
    count of slot g."""
    buckets = tids_flat.astype(np.int64) % N_BUCKETS
    order = np.argsort(buckets, kind='stable')
    counts = np.bincount(buckets, minlength=N_BUCKETS)
    cum = np.concatenate([[0], np.cumsum(counts)])
    tok_of_bucket = {b: order[cum[b]:cum[b + 1]]
                     for b in range(N_BUCKETS) if counts[b] > 0}

    n_bins = NGRP * N_CORES
    occ = sorted(tok_of_bucket, key=lambda b: -counts[b])
    bin_rows = np.zeros(n_bins, np.int64)
    bin_cnt = np.zeros(n_bins, np.int64)
    bin_members = [[] for _ in range(n_bins)]
    for b in occ:
        cand = np.where(bin_cnt < BPG)[0]
        i = cand[np.argmin(bin_rows[cand])]
        bin_members[i].append(b)
        bin_rows[i] += counts[b]
        bin_cnt[i] += 1
    assert bin_rows.max() <= 128, "group row overflow"
    # sort bins by rows desc; slot g = bins[8g:8g+8]; snake over cores
    srt = np.argsort(-bin_rows, kind='stable')
    bins, NT = [], []
    for g in range(NGRP):
        sel = [srt[g * N_CORES + c] for c in range(N_CORES)]
        sl = [bin_members[i] for i in sel]
        if g % 2:
            sl = sl[::-1]
        bins.append(sl)
        mx = max(2, max(bin_rows[i] for i in sel))
        NT.append(int(min(128, (mx + 1) // 2 * 2)))
    return bins, NT, tok_of_bucket


def _geom(NT):
    """Per-group kv column geometry: KCH chunks of [K^T_k | a^T_k]
    (NSP cols each), oh8h block (NT[g] cols), then 2 V halves (D each)."""
    wgs = [KCH * NSP + NT[g] + 2 * D for g in range(NGRP)]
    kcols = np.concatenate([[0], np.cumsum(wgs)]).astype(int)
    rows = np.concatenate([[0], np.cumsum(NT)]).astype(int)
    qtc = np.concatenate([[0], np.cumsum([KCH * n for n in NT])]).astype(int)
    return wgs, kcols, rows, qtc


def _consts():
    iota8 = np.broadcast_to(np.arange(BPG, dtype=np.float32),
                            (128, BPG)).copy()
    win = (np.arange(NS)[None, :] // SPB) == np.arange(BPG)[:, None]
    winmask8 = np.where(win, 0.0, NEG).astype(np.float32)      # [BPG, NS]
    identw = np.eye(128, dtype=np.float16)
    return iota8, winmask8, identw


def _pack_core(core_bins, NT, tok_of_bucket, q_flat, tids_flat,
               KT, V, slot_tids, CBT):
    """Build this core's input arrays. core_bins[g] = list of bucket ids."""
    wgs, kcols, rows, qtc = _geom(NT)
    i16 = int(rows[-1])
    qrow = np.zeros((i16, D), np.float16)
    qT = np.zeros((128, int(qtc[-1])), np.float16)
    side = np.full((i16, 2), -1.0, np.float32)
    tidb = np.full((NGRP, NS), -2.0, np.float32)
    kv = np.zeros((128, int(kcols[-1])), np.float16)
    tok_idx = np.full(i16, -1, np.int64)

    for g in range(NGRP):
        nt, col = NT[g], int(kcols[g])
        slot_ids = np.zeros(NS, np.int64)
        real_slots = np.zeros(NS, bool)
        anchors = np.zeros((D, BPG), np.float32)
        oh8h = np.zeros((BPG, nt), np.float32)
        qTv = qT[:, int(qtc[g]):int(qtc[g]) + KCH * nt].reshape(128, KCH, nt)
        r = 0
        for j, b in enumerate(core_bins[g]):
            toks = tok_of_bucket[b]
            c = len(toks)
            slot_ids[j * SPB:(j + 1) * SPB] = np.arange(b * SPB, (b + 1) * SPB)
            real_slots[j * SPB:(j + 1) * SPB] = True
            anchors[:, j] = CBT[:, b]
            tidb[g, j * SPB:(j + 1) * SPB] = \
                slot_tids[b * SPB:(b + 1) * SPB] + j * WENC
            r0 = int(rows[g]) + r
            qg = q_flat[toks]
            qrow[r0:r0 + c] = qg
            qTv[:, :, r:r + c] = qg.reshape(c, KCH, 128).transpose(2, 1, 0)
            side[r0:r0 + c, 0] = tids_flat[toks] + j * WENC
            side[r0:r0 + c, 1] = j
            oh8h[j, r:r + c] = 0.5
            tok_idx[r0:r0 + c] = toks
            r += c
        # ka chunks [KCH, 128, NSP]: per chunk K^T slots then anchors
        ktg = KT[:, slot_ids].reshape(KCH, 128, NS) * real_slots[None, None, :]
        atp = anchors.reshape(KCH, 128, BPG)
        ka = np.concatenate([ktg, atp], axis=2)
        c0 = col
        kv[:, c0:c0 + KCH * NSP] = \
            ka.transpose(1, 0, 2).reshape(128, -1).astype(np.float16)
        c0 += KCH * NSP
        kv[0:BPG, c0:c0 + nt] = oh8h.astype(np.float16)
        c0 += nt
        vb = V[slot_ids] * real_slots[:, None]          # [NS, D]
        kv[:, c0:c0 + D] = vb[0:128].astype(np.float16)
        kv[:, c0 + D:c0 + 2 * D] = vb[128:256].astype(np.float16)
    return dict(qrow=qrow, qT=qT, side=side, tidb=tidb, kv=kv), tok_idx


# ------------------------------------------------------------- device kernel

def _build_nc(NT):
    from concourse import bacc, mybir, tile

    F32 = mybir.dt.float32
    F16 = mybir.dt.float16
    AL = mybir.AluOpType
    AF = mybir.ActivationFunctionType
    X = mybir.AxisListType.X

    wgs, kcols, rows, qtc = _geom(NT)
    i16 = int(rows[-1])
    wmax = max(wgs)
    ntmax = max(NT)

    nc = bacc.Bacc(trn_type="TRN2", target_bir_lowering=False, debug=False)
    d_kv = nc.dram_tensor("kv", [128, int(kcols[-1])], F16,
                          kind="ExternalInput").ap()
    d_qT = nc.dram_tensor("qT", [128, int(qtc[-1])], F16,
                          kind="ExternalInput").ap()
    d_qrow = nc.dram_tensor("qrow", [i16, D], F16, kind="ExternalInput").ap()
    d_side = nc.dram_tensor("side", [i16, 2], F32, kind="ExternalInput").ap()
    d_tidb = nc.dram_tensor("tidb", [NGRP, NS], F32, kind="ExternalInput").ap()
    d_iota8 = nc.dram_tensor("iota8", [128, BPG], F32,
                             kind="ExternalInput").ap()
    d_winmask8 = nc.dram_tensor("winmask8", [BPG, NS], F32,
                                kind="ExternalInput").ap()
    d_identw = nc.dram_tensor("identw", [128, 128], F16,
                              kind="ExternalInput").ap()
    d_out = nc.dram_tensor("outp", [i16, D], F16, kind="ExternalOutput").ap()

    with tile.TileContext(nc) as tc:
        with tc.tile_pool(name="const", bufs=1) as pc, \
             tc.tile_pool(name="kvp", bufs=4) as pkv, \
             tc.tile_pool(name="io", bufs=4) as pio, \
             tc.tile_pool(name="wk", bufs=2) as pw, \
             tc.tile_pool(name="outp", bufs=3) as po, \
             tc.tile_pool(name="psA", bufs=2, space="PSUM") as ppa, \
             tc.tile_pool(name="psB", bufs=1, space="PSUM") as ppb:

            iota8 = pc.tile([128, BPG], F32)
            winmask8 = pc.tile([BPG, NS], F32)
            identw = pc.tile([128, 128], F16)
            nc.sync.dma_start(iota8[:], d_iota8)
            nc.sync.dma_start(winmask8[:], d_winmask8)
            nc.sync.dma_start(identw[:], d_identw)
            eps6 = pc.tile([128, 1], F32)
            nc.gpsimd.memset(eps6[:], 1e-6)
            half05 = pc.tile([128, 1], F32)
            nc.gpsimd.memset(half05[:], 0.5)
            ones256 = pc.tile([128, NS], F32)
            nc.gpsimd.memset(ones256[:], 1.0)

            for g in range(NGRP):
                nt = NT[g]
                wg = wgs[g]
                col = int(kcols[g])
                r0 = int(rows[g])
                qc = int(qtc[g])
                voff = KCH * NSP + nt   # V offset within this group's kv

                kv_t = pkv.tile([128, wmax], F16, tag="kv")
                nc.sync.dma_start(kv_t[:, 0:wg], d_kv[:, col:col + wg])
                ka = kv_t[:, 0:KCH * NSP].rearrange("p (k s) -> p k s", k=KCH)
                qt_t = pio.tile([128, KCH * ntmax], F16, tag="qt")
                nc.scalar.dma_start(qt_t[:, 0:KCH * nt],
                                    d_qT[:, qc:qc + KCH * nt])
                qt = qt_t[:, 0:KCH * nt].rearrange("p (k t) -> p k t", k=KCH)
                qr_t = pio.tile([128, D], F16, tag="qr")
                nc.scalar.dma_start(qr_t[0:nt, :], d_qrow[r0:r0 + nt, :])
                side_t = pio.tile([128, 2], F32, tag="side")
                nc.sync.dma_start(side_t[0:nt, :], d_side[r0:r0 + nt, :])
                tidb_t = pio.tile([128, NS], F32, tag="tidb")
                nc.sync.dma_start(tidb_t[0:nt, :],
                                  d_tidb[g:g + 1, :].to_broadcast((nt, NS)))

                # --- mask8: row's own instance column (for q.a extraction)
                mask8 = pw.tile([128, BPG], F32, tag="mask8")
                nc.vector.tensor_scalar(out=mask8[0:nt, :],
                                        in0=iota8[0:nt, :],
                                        scalar1=side_t[0:nt, 1:2],
                                        scalar2=None, op0=AL.is_equal)

                # --- ssq = |q|^2 ; s1 = 0.5/|q| = rsqrt(4 ssq)
                scr = pw.tile([128, D], F16, tag="scr")
                ssq = pw.tile([128, 1], F32, tag="ssq")
                nc.vector.scalar_tensor_tensor(
                    out=scr[0:nt, :], in0=qr_t[0:nt, :], scalar=1.0,
                    in1=qr_t[0:nt, :], op0=AL.mult, op1=AL.mult,
                    accum_out=ssq[0:nt, :])
                s1n = pw.tile([128, 1], F32, tag="s1n")
                nc.scalar.activation(s1n[0:nt, :], ssq[0:nt, :], AF.Sqrt,
                                     bias=eps6[0:nt, :], scale=4.0)
                s1 = pw.tile([128, 1], F32, tag="s1")
                nc.vector.reciprocal(s1[0:nt, :], s1n[0:nt, :])

                # --- anchor-dot table a0t = a.K + winmask  [BPG, NS]
                a0t_ps = ppb.tile([BPG, NS], F32, tag="a0t")
                for k in range(KCH):
                    nc.tensor.matmul(a0t_ps[:], ka[:, k, NS:NSP],
                                     ka[:, k, 0:NS], start=(k == 0),
                                     stop=(k == KCH - 1))
                a0t = pw.tile([BPG, NS], F16, tag="a0tsb")
                nc.vector.scalar_tensor_tensor(
                    out=a0t[:], in0=a0t_ps[:], scalar=1.0,
                    in1=winmask8[:], op0=AL.mult, op1=AL.add)

                # --- u = [q.K | q.a] ; c = 0.5*(a.K + winmask) per row
                u_ps = ppa.tile([128, NSP], F32, tag="u")
                for k in range(KCH):
                    nc.tensor.matmul(u_ps[0:nt, :], qt[:, k, 0:nt],
                                     ka[:, k, :], start=(k == 0),
                                     stop=(k == KCH - 1))
                c_ps = ppa.tile([128, NS], F32, tag="c")
                nc.tensor.matmul(c_ps[0:nt, :],
                                 kv_t[0:BPG, KCH * NSP:KCH * NSP + nt],
                                 a0t[:], start=True, stop=True)

                # --- rw10 = (1/W)/tau = rsqrt(W^2 * tau^2),  W^2 = 0.5+qa*s1
                qasc = pw.tile([128, BPG], F32, tag="qasc")
                qa1 = pw.tile([128, 1], F32, tag="qa1")
                nc.vector.scalar_tensor_tensor(
                    out=qasc[0:nt, :], in0=u_ps[0:nt, NS:NSP], scalar=1.0,
                    in1=mask8[0:nt, :], op0=AL.mult, op1=AL.mult,
                    accum_out=qa1[0:nt, :])
                w2 = pw.tile([128, 1], F32, tag="w2")
                nc.vector.scalar_tensor_tensor(
                    out=w2[0:nt, :], in0=qa1[0:nt, :], scalar=s1[0:nt, :],
                    in1=half05[0:nt, :], op0=AL.mult, op1=AL.add)
                rwn = pw.tile([128, 1], F32, tag="rwn")
                nc.scalar.activation(rwn[0:nt, :], w2[0:nt, :], AF.Sqrt,
                                     scale=TAU * TAU)
                rw10 = pw.tile([128, 1], F32, tag="rw10")
                nc.vector.reciprocal(rw10[0:nt, :], rwn[0:nt, :])

                # --- sc = u*s1 + c   (masked cols ~ -15000)
                sc1 = pw.tile([128, NS], F32, tag="sc1")
                nc.vector.tensor_scalar(out=sc1[0:nt, :],
                                        in0=u_ps[0:nt, 0:NS],
                                        scalar1=s1[0:nt, :], scalar2=None,
                                        op0=AL.mult)
                sc = pw.tile([128, NS], F32, tag="sc")
                nc.vector.tensor_tensor(out=sc[0:nt, :], in0=sc1[0:nt, :],
                                        in1=c_ps[0:nt, :], op=AL.add)
                negmax = pw.tile([128, 1], F32, tag="negmax")
                nc.vector.reduce_max(negmax[0:nt, :], sc[0:nt, :], axis=X,
                                     negate=True)
                ebias = pw.tile([128, 1], F32, tag="ebias")
                nc.vector.tensor_tensor(out=ebias[0:nt, :],
                                        in0=negmax[0:nt, :],
                                        in1=rw10[0:nt, :], op=AL.mult)
                ex = pw.tile([128, NS], F32, tag="ex")
                esum = pw.tile([128, 1], F32, tag="esum")
                nc.scalar.activation(ex[0:nt, :], sc[0:nt, :], AF.Exp,
                                     bias=ebias[0:nt, :], scale=rw10[0:nt, :],
                                     accum_out=esum[0:nt, :])
                rsum = pw.tile([128, 1], F32, tag="rsum")
                nc.vector.reciprocal(rsum[0:nt, :], esum[0:nt, :])

                # --- hard match path (window folded into tid encoding)
                match = pw.tile([128, NS], F32, tag="match")
                msum = pw.tile([128, 1], F32, tag="msum")
                nc.vector.scalar_tensor_tensor(
                    out=match[0:nt, :], in0=tidb_t[0:nt, :],
                    scalar=side_t[0:nt, 0:1], in1=ones256[0:nt, :],
                    op0=AL.is_equal, op1=AL.mult,
                    accum_out=msum[0:nt, :])
                mden = pw.tile([128, 1], F32, tag="mden")
                nc.vector.tensor_scalar(out=mden[0:nt, :], in0=msum[0:nt, :],
                                        scalar1=1e-9, scalar2=None, op0=AL.add)
                mrec = pw.tile([128, 1], F32, tag="mrec")
                nc.vector.reciprocal(mrec[0:nt, :], mden[0:nt, :])
                nohas = pw.tile([128, 1], F32, tag="nohas")
                nc.vector.tensor_scalar(out=nohas[0:nt, :], in0=msum[0:nt, :],
                                        scalar1=0.0, scalar2=None,
                                        op0=AL.is_le)
                rs_nh = pw.tile([128, 1], F32, tag="rs_nh")
                nc.vector.tensor_tensor(out=rs_nh[0:nt, :], in0=rsum[0:nt, :],
                                        in1=nohas[0:nt, :], op=AL.mult)
                hard = pw.tile([128, NS], F32, tag="hard")
                nc.vector.tensor_scalar(out=hard[0:nt, :],
                                        in0=match[0:nt, :],
                                        scalar1=mrec[0:nt, :], scalar2=None,
                                        op0=AL.mult)
                probs = pw.tile([128, NS], F16, tag="probs")
                nc.vector.scalar_tensor_tensor(
                    out=probs[0:nt, :], in0=ex[0:nt, :],
                    scalar=rs_nh[0:nt, :], in1=hard[0:nt, :],
                    op0=AL.mult, op1=AL.add)

                # --- probs^T, then val = probs @ V
                pt_ps = ppb.tile([128, 2, 128], F16, tag="pt")
                for h in range(2):
                    nc.tensor.transpose(pt_ps[:, h, 0:nt],
                                        probs[0:nt, h * 128:(h + 1) * 128],
                                        identw[0:nt, 0:nt])
                pt = pw.tile([128, 2, 128], F16, tag="ptsb")
                for h in range(2):
                    nc.vector.tensor_copy(pt[:, h, 0:nt], pt_ps[:, h, 0:nt])
                pv = ppb.tile([128, D], F32, tag="pv")
                for j in range(2):
                    for h in range(2):
                        nc.tensor.matmul(
                            pv[0:nt, j * 512:(j + 1) * 512],
                            pt[:, h, 0:nt],
                            kv_t[:, voff + h * D + j * 512:
                                 voff + h * D + (j + 1) * 512],
                            start=(h == 0), stop=(h == 1))
                out_sb = po.tile([128, D], F16, tag="out_sb")
                nc.vector.tensor_copy(out_sb[0:nt, :], pv[0:nt, :])
                nc.scalar.dma_start(d_out[r0:r0 + nt, :], out_sb[0:nt, :])
    nc.compile()
    return nc


# ------------------------------------------------------------------ emulator

def _emulate_core(ins, NT):
    """Numpy emulation of the device kernel (fp16 data, f32 math)."""
    qrow = ins["qrow"].astype(np.float32)
    qT = ins["qT"].astype(np.float32)
    side, tidb = ins["side"], ins["tidb"]
    kv = ins["kv"].astype(np.float32)
    wgs, kcols, rows, qtc = _geom(NT)
    i16 = int(rows[-1])
    out = np.zeros((i16, D), np.float32)
    iota8, winmask8, _ = _consts()
    for g in range(NGRP):
        nt, col = NT[g], int(kcols[g])
        voff = col + KCH * NSP + nt
        ka = kv[:, col:col + KCH * NSP].reshape(128, KCH, NSP)
        KT = ka[:, :, 0:NS].transpose(1, 0, 2).reshape(D, NS)
        AT = ka[:, :, NS:NSP].transpose(1, 0, 2).reshape(D, BPG)
        oh8h = kv[0:BPG, col + KCH * NSP:col + KCH * NSP + nt]
        vb = np.concatenate([kv[:, voff:voff + D].T,
                             kv[:, voff + D:voff + 2 * D].T], axis=1).T
        q = qrow[rows[g]:rows[g] + nt]
        qTg = qT[:, int(qtc[g]):int(qtc[g]) + KCH * nt].reshape(128, KCH, nt)
        qTg = qTg.transpose(1, 0, 2).reshape(D, nt)
        sd = side[rows[g]:rows[g] + nt]

        mask8 = (iota8[0:nt, :] == sd[:, 1:2]).astype(np.float32)
        ssq = (q * q).sum(-1, keepdims=True)
        s1 = 1.0 / np.sqrt(4.0 * ssq + 1e-6)
        a0t = (AT.T @ KT + winmask8).astype(np.float16).astype(np.float32)
        u = qTg.T @ np.concatenate([KT, AT], axis=1)    # [nt, NSP]
        c = oh8h.T @ a0t
        qa1 = (u[:, NS:NSP] * mask8).sum(-1, keepdims=True)
        w2 = qa1 * s1 + 0.5
        rw10 = 1.0 / np.sqrt(w2 * TAU * TAU)
        sc = u[:, 0:NS] * s1 + c
        m = sc.max(-1, keepdims=True)
        ex = np.exp((sc - m) * rw10)
        esum = ex.sum(-1, keepdims=True)
        match = (tidb[g][None, :] == sd[:, 0:1]).astype(np.float32)
        msum = match.sum(-1, keepdims=True)
        nohas = (msum <= 0).astype(np.float32)
        hard = match / (msum + 1e-9)
        probs = (ex * (nohas / esum) + hard).astype(np.float16)
        out[rows[g]:rows[g] + nt] = \
            (probs.astype(np.float32) @ vb).astype(np.float16)
    return out


# -------------------------------------------------------------------- kernel

def kernel(query_emb, tids, slot_keys, slot_values, slot_tids,
           centroid_codebook, _emulate=False, _trace=False):
    B, T, _ = query_emb.shape
    BT = B * T
    q_flat = np.ascontiguousarray(query_emb.reshape(BT, D), np.float32)
    tids_flat = np.asarray(tids).reshape(BT)
    st = np.asarray(slot_tids).astype(np.float32)
    KT = np.ascontiguousarray(np.asarray(slot_keys, np.float32).T)     # [D, S]
    V = np.asarray(slot_values, np.float32)
    CBT = np.ascontiguousarray(np.asarray(centroid_codebook, np.float32).T)

    bins, NT, tok_of_bucket = _routing(tids_flat)
    iota8, winmask8, identw = _consts()

    in_maps, tok_idxs = [], []
    for c in range(N_CORES):
        core_bins = [bins[g][c] for g in range(NGRP)]
        ins, tok_idx = _pack_core(core_bins, NT, tok_of_bucket, q_flat,
                                  tids_flat, KT, V, st, CBT)
        ins.update(iota8=iota8, winmask8=winmask8, identw=identw)
        in_maps.append(ins)
        tok_idxs.append(tok_idx)

    out_flat = np.zeros((BT, D), np.float32)
    if _emulate:
        for c in range(N_CORES):
            o = _emulate_core(in_maps[c], NT)
            valid = tok_idxs[c] >= 0
            out_flat[tok_idxs[c][valid]] = o[valid]
        return out_flat.reshape(B, T, D).astype(np.float32)

    _install_ntff_hook()
    from concourse import bass_utils
    key = tuple(NT)
    if key not in _COMPILED:
        _COMPILED[key] = _build_nc(NT)
    nc = _COMPILED[key]
    res = bass_utils.run_bass_kernel_spmd(
        nc, in_maps, core_ids=list(range(N_CORES)), trace=_trace)
    for c in range(N_CORES):
        o = np.asarray(res.results[c]["outp"], np.float32)
        valid = tok_idxs[c] >= 0
        out_flat[tok_idxs[c][valid]] = o[valid]
    out = out_flat.reshape(B, T, D).astype(np.float32)
    if _trace:
        kernel._last_exec_time_ns = res.exec_time_ns
        kernel._last_results = res
    return out
